# revision 1
# baseline (speedup 1.0000x reference)
"""Cross-attention fusion kernel for Trainium2, 8-way SPMD.

Sharding: the N=4096 attention query rows are split 512/core (= 8 rows of the
64x64 downsampled grid = 32 rows of the 256x256 output). conv_down runs on a
per-core input band; x2d/x3d shards are AllGathered (bf16) so each core holds
the full K/V source. The attention is computed transposed (attnT[j,i]) so no
on-device transposes are needed. conv_transpose + channel-concat + 1x1 fuse
conv are folded into a single matmul family via host-precomputed combined
weights (Wcomb = einsum(w_up, w_fuse)).
"""
import numpy as np
import ml_dtypes

import concourse.bacc as bacc
import concourse.mybir as mybir
import concourse.tile as tile
from concourse.bass_utils import run_bass_kernel_spmd

NCORES = 8
C = 256          # channels
CH = 2           # channel tiles of 128
HID = 128        # q/k hidden
R = 4            # stride
H = 256          # input H/W
HD = 64          # downsampled H/W
N = HD * HD      # 4096
RD = HD // NCORES   # x_d rows per core: 8
NL = RD * HD        # local attention rows: 512
NJT = N // 128      # j tiles: 32
BROWS = 4 * RD - 1  # conv band rows: 31
BROWS1 = BROWS + 3  # x1 band rows (conv + fuse halo): 34
BCOLS = H + 2       # padded cols: 258
SCALE = float(HID) ** -0.5

BF = mybir.dt.bfloat16
F32 = mybir.dt.float32

_CACHE = {}


def _build_nc(sim=False, ablate=()):
    nc = bacc.Bacc("TRN2", target_bir_lowering=False, debug=False,
                   enable_asserts=False,
                   num_devices=1 if sim else NCORES)

    def inp(name, shape, dt=BF):
        return nc.dram_tensor(name, shape, dt, kind="ExternalInput").ap()

    x1b = inp("x1b", [128, CH, BROWS1, BCOLS])
    x2b = inp("x2b", [128, CH, 24, 192])
    x3b = inp("x3b", [128, CH, 24, 192])
    wdT = inp("wdT", [128, CH, 9, C])
    wqT = inp("wqT", [128, CH, HID])
    wkT = inp("wkT", [128, CH, HID])
    wvT = inp("wvT", [128, CH, C])
    wca = inp("wca", [128, CH, 9, C])
    wcb = inp("wcb", [128, CH, 9, C])
    wfc = inp("wfc", [128, CH, C])
    bdown = inp("bdown", [128, CH], F32)
    bq = inp("bq", [128, 1], F32)
    bk = inp("bk", [128, 1], F32)
    bv128 = inp("bv128", [128, C], F32)
    beff = inp("beff", [128, CH], F32)
    onesb = inp("onesb", [128, 1], BF)
    ones1f = inp("ones1f", [1, 128], F32)
    ones128f = inp("ones128f", [128, 1], F32)

    out = nc.dram_tensor("out", [CH, 128, 4 * RD, H], F32,
                         kind="ExternalOutput").ap()

    # collective buffers (internal DRAM)
    x2d_sh = nc.dram_tensor("x2d_sh", [C, NL], BF).ap()
    x3d_sh = nc.dram_tensor("x3d_sh", [C, NL], BF).ap()
    x2d_fl = nc.dram_tensor("x2d_fl", [NCORES * C, NL], BF,
                            addr_space="Shared").ap()
    x3d_fl = nc.dram_tensor("x3d_fl", [NCORES * C, NL], BF,
                            addr_space="Shared").ap()
    rg = [list(range(NCORES))]

    with tile.TileContext(nc) as tc:
        with (
            tc.tile_pool(name="band", bufs=3) as band_pool,
            tc.tile_pool(name="bigw", bufs=2) as bigw_pool,
            tc.tile_pool(name="wsm", bufs=1) as wsm_pool,
            tc.tile_pool(name="xd", bufs=3) as xd_pool,
            tc.tile_pool(name="xdc", bufs=3) as xdc_pool,
            tc.tile_pool(name="kf", bufs=2) as kf_pool,
            tc.tile_pool(name="vt", bufs=1) as vt_pool,
            tc.tile_pool(name="small", bufs=1) as small_pool,
            tc.tile_pool(name="t", bufs=6) as t_pool,
            tc.tile_pool(name="ps", bufs=5, space="PSUM") as ps_pool,
            tc.tile_pool(name="psf", bufs=2, space="PSUM") as psf_pool,
            tc.tile_pool(name="pss", bufs=1, space="PSUM") as pss_pool,
        ):
            # ---- small weights resident for the whole kernel ----
            def load(pool, ap, tag, dt=None, split=1):
                t_ = pool.tile(ap.shape, dt or ap.dtype, tag=tag)
                if split == 1:
                    nc.sync.dma_start(out=t_[:], in_=ap[:])
                else:
                    # chunk dim 1 across DMA queues
                    d1 = ap.shape[1]
                    step = max(1, d1 // split)
                    for i in range(0, d1, step):
                        j = min(d1, i + step)
                        nc.sync.dma_start(out=t_[:, i:j], in_=ap[:, i:j])
                return t_

            wqT_s = load(wsm_pool, wqT, "wqT")
            wkT_s = load(wsm_pool, wkT, "wkT")
            wvT_s = load(wsm_pool, wvT, "wvT")
            wfc_s = load(wsm_pool, wfc, "wfc")
            bdown_s = load(wsm_pool, bdown, "bdown")
            bq_s = load(wsm_pool, bq, "bq")
            bk_s = load(wsm_pool, bk, "bk")
            bv_s = load(wsm_pool, bv128, "bv")
            beff_s = load(wsm_pool, beff, "beff")
            onesb_s = load(wsm_pool, onesb, "onesb")
            ones1f_s = load(wsm_pool, ones1f, "ones1f")
            ones128f_s = load(wsm_pool, ones128f, "o128f")
            wdT_s = load(bigw_pool, wdT, "bw", split=2)

            # ---- conv_down: band [128,CH,rows,258] -> xd [128,CH,NL] ----
            def conv_down(band_s, name, st=3):
                xd_s = xd_pool.tile([128, CH, NL], BF, tag="xd", name=name)
                for m in range(CH):
                    ps = ps_pool.tile([128, NL], F32, tag="ps")
                    first = True
                    for k in range(CH):
                        for dy in range(3):
                            for dx in range(3):
                                tap = dy * 3 + dx
                                rhs = band_s[:, k,
                                             dy:dy + 7 * st + 1:st,
                                             dx:dx + 63 * st + 1:st]
                                lhsT = wdT_s[:, k, tap, m * 128:(m + 1) * 128]
                                last = (k == CH - 1 and tap == 8)
                                nc.tensor.matmul(ps[:], lhsT=lhsT, rhs=rhs,
                                                 start=first, stop=last)
                                first = False
                    nc.vector.tensor_scalar_add(xd_s[:, m, :], ps[:],
                                                bdown_s[:, m:m + 1])
                return xd_s

            # x2 / x3 shards -> bounce -> AllGather
            for band_ap, sh, fl, name in ((x2b, x2d_sh, x2d_fl, "x2d"),
                                          (x3b, x3d_sh, x3d_fl, "x3d")):
                band_s = band_pool.tile(band_ap.shape, BF, tag="band",
                                        name=f"{name}b")
                for k in range(CH):
                    for ci, i in enumerate(range(0, 24, 6)):
                        eng = nc.sync if (k * 4 + ci) % 2 == 0 else nc.gpsimd
                        eng.dma_start(out=band_s[:, k, i:i + 6, :],
                                      in_=band_ap[:, k, i:i + 6, :])
                xd_s = conv_down(band_s, name)
                shv = sh.rearrange("(h p) n -> h p n", h=CH)
                for m in range(CH):
                    nc.sync.dma_start(out=shv[m], in_=xd_s[:, m, :])
                if sim:
                    # collective-free stand-in for TimelineSim: local copy
                    nc.sync.dma_start(out=fl[0:C, :], in_=sh[:])
                else:
                    nc.gpsimd.collective_compute(
                        "AllGather", mybir.AluOpType.bypass, replica_groups=rg,
                        ins=[sh[:]], outs=[fl[:]])

            # x1 band -> x1d -> q
            x1b_s = band_pool.tile(x1b.shape, BF, tag="band", name="x1bb")
            for k in range(CH):
                for ci, i in enumerate(range(0, BROWS1, 5)):
                    j = min(BROWS1, i + 5)
                    eng = nc.sync if (k * 7 + ci) % 2 == 0 else nc.gpsimd
                    eng.dma_start(out=x1b_s[:, k, i:j, :],
                                  in_=x1b[:, k, i:j, :])
            x1d_s = conv_down(x1b_s[:, :, 0:BROWS, :], "x1d", st=4)
            ps_q = ps_pool.tile([128, NL], F32, tag="ps")
            for k in range(CH):
                nc.tensor.matmul(ps_q[:], lhsT=wqT_s[:, k, :],
                                 rhs=x1d_s[:, k, :],
                                 start=(k == 0), stop=(k == CH - 1))
            qf_s = small_pool.tile([128, NL], BF, tag="qf")
            nc.vector.tensor_scalar_add(qf_s[:], ps_q[:], bq_s[:])

            # ---- attends ----
            feat_s = small_pool.tile([128, 2, CH, NL], BF, tag="feat")

            attend_srcs = () if "noattend" in ablate else (x2d_fl, x3d_fl)
            for ei, fl in enumerate(attend_srcs):
                flv = fl.rearrange("(r h p) n -> h r p n", r=NCORES, h=CH)
                kf_s = kf_pool.tile([128, N], BF, tag="kf")
                vt_s = vt_pool.tile([128, NJT, C], BF, tag="vt")
                for n in range(NCORES):
                    # stream full x_d chunk n: [128, CH, NL]
                    xc = xdc_pool.tile([128, CH, NL], BF, tag="xdc")
                    for k in range(CH):
                        eng = nc.sync if (n + k) % 2 == 0 else nc.gpsimd
                        eng.dma_start(out=xc[:, k, :], in_=flv[k, n])
                    # kf chunk
                    ps_k = ps_pool.tile([128, NL], F32, tag="ps")
                    for k in range(CH):
                        nc.tensor.matmul(ps_k[:], lhsT=wkT_s[:, k, :],
                                         rhs=xc[:, k, :],
                                         start=(k == 0), stop=(k == CH - 1))
                    nc.vector.tensor_scalar_add(kf_s[:, n * NL:(n + 1) * NL],
                                                ps_k[:], bk_s[:])
                    # vT tiles for this chunk
                    for j in range(4):
                        jt = n * 4 + j
                        ps_v = ps_pool.tile([128, C], F32, tag="ps")
                        for k in range(CH):
                            nc.tensor.matmul(
                                ps_v[:],
                                lhsT=xc[:, k, j * 128:(j + 1) * 128],
                                rhs=wvT_s[:, k, :],
                                start=(k == 0), stop=(k == CH - 1))
                        nc.vector.tensor_add(vt_s[:, jt, :], ps_v[:], bv_s[:])

                # attention
                ps_f = [psf_pool.tile([128, NL], F32, tag="psf",
                                      name=f"psf{ei}_{m}")
                        for m in range(CH)]
                ps_s = pss_pool.tile([1, NL], F32, tag="pss")
                acc_s = small_pool.tile([128, NL], F32, tag="acc",
                                        name=f"acc{ei}")
                for jt in range(NJT):
                    ps_a = ps_pool.tile([128, NL], F32, tag="ps")
                    nc.tensor.matmul(ps_a[:],
                                     lhsT=kf_s[:, jt * 128:(jt + 1) * 128],
                                     rhs=qf_s[:], start=True, stop=True)
                    t_s = t_pool.tile([128, NL], BF, tag="t")
                    nc.scalar.activation(t_s[:], ps_a[:],
                                         mybir.ActivationFunctionType.Exp,
                                         scale=SCALE)
                    for m in range(CH):
                        nc.tensor.matmul(ps_f[m][:],
                                         lhsT=vt_s[:, jt, m * 128:(m + 1) * 128],
                                         rhs=t_s[:],
                                         start=(jt == 0), stop=(jt == NJT - 1))
                    # partial softmax denominator on DVE (per-partition)
                    if jt == 0:
                        nc.vector.tensor_copy(acc_s[:], t_s[:])
                    else:
                        nc.vector.tensor_add(acc_s[:], acc_s[:], t_s[:])
                # single cross-partition reduction of the accumulated sums
                nc.tensor.matmul(ps_s[:], lhsT=ones128f_s[:], rhs=acc_s[:],
                                 start=True, stop=True)

                # normalize: r = 1/s broadcast to 128 partitions via matmul
                r_s = small_pool.tile([1, NL], F32, tag="rs")
                nc.vector.reciprocal(r_s[:], ps_s[:])
                ps_r = ps_pool.tile([128, NL], F32, tag="ps")
                nc.tensor.matmul(ps_r[:], lhsT=ones1f_s[:], rhs=r_s[:],
                                 start=True, stop=True)
                rb_s = small_pool.tile([128, NL], F32, tag="rb")
                nc.vector.tensor_copy(rb_s[:], ps_r[:])
                for m in range(CH):
                    nc.vector.tensor_mul(feat_s[:, ei, m, :], ps_f[m][:],
                                         rb_s[:])

            if "nofuse" not in ablate:
                # ---- fused convT + concat + 1x1 fuse conv ----
                wca_s = load(bigw_pool, wca, "bw", split=2)
                wcb_s = load(bigw_pool, wcb, "bw", split=2)
                # two row-halves: y' in [0,4) and [4,8)
                for half in range(2):
                    y0 = half * 4
                    stg = band_pool.tile([128, CH, 2 * RD, H], F32, tag="band",
                                         name=f"stg{half}")
                    sgs = [(ky, kx) for ky in range(4) for kx in range(4)]
                    sgs.sort(key=lambda p: (p[0] < 3 and p[1] < 3))
                    for ky, kx in sgs:
                        for m in range(CH):
                                ps_o = ps_pool.tile([128, 4, HD], F32, tag="ps")
                                first = True
                                if ky < 3 and kx < 3:
                                    tap = ky * 3 + kx
                                    for ws, e in ((wca_s, 0), (wcb_s, 1)):
                                        for k in range(CH):
                                            nc.tensor.matmul(
                                                ps_o[:],
                                                lhsT=ws[:, k, tap,
                                                        m * 128:(m + 1) * 128],
                                                rhs=feat_s[:, e, k,
                                                           y0 * HD:(y0 + 4) * HD],
                                                start=first, stop=False)
                                            first = False
                                for k in range(CH):
                                    rhs = x1b_s[:, k,
                                                4 * y0 + ky + 1:4 * y0 + ky + 14:4,
                                                kx + 1:kx + 254:4]
                                    nc.tensor.matmul(ps_o[:],
                                                     lhsT=wfc_s[:, k,
                                                                m * 128:(m + 1) * 128],
                                                     rhs=rhs,
                                                     start=first,
                                                     stop=(k == CH - 1))
                                    first = False
                                nc.vector.tensor_scalar_add(
                                    stg[:, m, ky:ky + 13:4, kx:kx + 253:4],
                                    ps_o[:], beff_s[:, m:m + 1])
                    ov = out.rearrange("h p (g y) x -> g h p y x", g=2)
                    for m in range(CH):
                        for ci, i in enumerate(range(0, 2 * RD, 4)):
                            eng = nc.sync if (m * 4 + ci) % 2 == 0 else nc.gpsimd
                            eng.dma_start(
                                out=ov[half, m][:, i:i + 4, :],
                                in_=stg[:, m, i:i + 4, :])

    nc.compile()
    return nc


def _prep_inputs(x1, x2, x3, w_down, b_down, w_q, b_q, w_k, b_k, w_v, b_v,
                 w_up, b_up, w_fuse, b_fuse):
    bf = ml_dtypes.bfloat16

    def to_tiles(a):
        # [C, ...] -> [128, CH, ...]
        return np.ascontiguousarray(
            a.reshape(CH, 128, *a.shape[1:]).transpose(
                1, 0, *range(2, a.ndim + 1)))

    def band(x, r, nrows):
        # rows 32r-1 .. 32r-1+nrows-1, cols padded by 1 -> [128,CH,nrows,258]
        b = np.zeros((C, nrows, BCOLS), np.float32)
        lo = 32 * r - 1
        s0, s1 = max(0, lo), min(H, lo + nrows)
        b[:, s0 - lo:s1 - lo, 1:H + 1] = x[0, :, s0:s1, :]
        return to_tiles(b).astype(bf)

    rows24 = (np.arange(8)[:, None] * 4 + np.arange(3)).ravel()
    cols192 = (np.arange(64)[:, None] * 4 + np.arange(3)).ravel() - 1

    def band_packed(x, r):
        # only the rows/cols a stride-4 3x3 tap reads: [128,CH,24,192]
        rows = rows24 + 32 * r - 1
        rv = np.clip(rows, 0, H - 1)
        cv = np.clip(cols192, 0, H - 1)
        b = x[0][:, rv[:, None], cv[None, :]].astype(np.float32)
        b[:, rows < 0, :] = 0.0
        b[:, rows >= H, :] = 0.0
        b[:, :, cols192 < 0] = 0.0
        return to_tiles(b).astype(bf)

    wf = w_fuse[:, :, 0, 0]                      # [C, 3C]
    wdT = to_tiles(w_down.transpose(1, 2, 3, 0).reshape(C, 9, C)).astype(bf)
    wqT = to_tiles(w_q[:, :, 0, 0].T.copy()).astype(bf)
    wkT = to_tiles(w_k[:, :, 0, 0].T.copy()).astype(bf)
    wvT = to_tiles(w_v[:, :, 0, 0].T.copy()).astype(bf)
    wca = to_tiles(np.einsum('iokl,co->iklc', w_up, wf[:, :C],
                             optimize=True).reshape(C, 9, C)).astype(bf)
    wcb = to_tiles(np.einsum('iokl,co->iklc', w_up, wf[:, C:2 * C],
                             optimize=True).reshape(C, 9, C)).astype(bf)
    wfc = to_tiles(wf[:, 2 * C:].T.copy()).astype(bf)
    b_eff = (b_fuse + wf[:, :C] @ b_up + wf[:, C:2 * C] @ b_up)

    shared = {
        "wdT": wdT, "wqT": wqT, "wkT": wkT, "wvT": wvT,
        "wca": wca, "wcb": wcb, "wfc": wfc,
        "bdown": np.ascontiguousarray(b_down.reshape(CH, 128).T).astype(np.float32),
        "bq": b_q.reshape(128, 1).astype(np.float32),
        "bk": b_k.reshape(128, 1).astype(np.float32),
        "bv128": np.broadcast_to(b_v, (128, C)).copy().astype(np.float32),
        "beff": np.ascontiguousarray(b_eff.reshape(CH, 128).T).astype(np.float32),
        "onesb": np.ones((128, 1), bf),
        "ones1f": np.ones((1, 128), np.float32),
        "ones128f": np.ones((128, 1), np.float32),
    }
    in_maps = []
    for r in range(NCORES):
        m = dict(shared)
        m["x1b"] = band(x1, r, BROWS1)
        m["x2b"] = band_packed(x2, r)
        m["x3b"] = band_packed(x3, r)
        in_maps.append(m)
    return in_maps


def kernel(**inputs):
    inputs = {k: np.asarray(v) for k, v in inputs.items()}
    in_maps = _prep_inputs(**inputs)
    if "nc" not in _CACHE:
        _CACHE["nc"] = _build_nc()
    res = run_bass_kernel_spmd(_CACHE["nc"], in_maps,
                               core_ids=list(range(NCORES)))
    out = np.empty((1, C, H, H), np.float32)
    for r in range(NCORES):
        band = res.results[r]["out"].reshape(C, 4 * RD, H)
        out[0, :, 32 * r:32 * r + 32, :] = band
    return out



# revision 14
# speedup vs baseline: 1.7295x; 1.7295x over previous
"""Cross-attention fusion kernel for Trainium2, 8-way SPMD — fp8 DoubleRow.

Sharding: the N=4096 attention query rows are split 512/core. Each core
downsamples its own band of x1/x2/x3 (stride-4 3x3 conv as 9 fp8 DoubleRow
matmuls over packed tap bands), computes k/v projections for its own chunk
only, and AllGathers kf (HID-split [64,2,512] layout) + vT (fp8). The
attention runs entirely in fp8 DoubleRow: QK^T contracts HID as 2x64
partition tiles, attn@V contracts j as paired 128-tiles, and the softmax
denominator comes from one extra DoubleRow matmul with a ones lhsT. Biases
that cancel in softmax (b_down, b_k) are dropped; v/q biases are folded into
a host-precomputed per-tap output bias / q-channel constant. The
convT+concat+1x1-fuse stage uses fp8 DoubleRow for the two attention
branches and bf16 for the x1 path, staged in SBUF and streamed out.
Weights are pre-scaled by 32 on the host to center fp8 exponents; inverse
scales fold into PSUM-copy scale factors and the exp() activation scale.
"""
import numpy as np
import ml_dtypes

import concourse.bacc as bacc
import concourse.mybir as mybir
import concourse.tile as tile
from concourse.bass_utils import run_bass_kernel_spmd

NCORES = 8
C = 256          # channels
CH = 2           # channel tiles of 128
HID = 128        # q/k hidden
H = 256          # input H/W
HD = 64          # downsampled H/W
N = HD * HD      # 4096
RD = HD // NCORES   # x_d rows per core: 8
NL = RD * HD        # local attention rows: 512
NJT = N // 128      # j tiles: 32
WS = 32.0           # host weight pre-scale
SCALE = float(HID) ** -0.5
ESHIFT = -2.0       # exp(S + shift): cancels in softmax, guards fp8 range

BF = mybir.dt.bfloat16
F32 = mybir.dt.float32
F8 = mybir.dt.float8e4
DR = mybir.MatmulPerfMode.DoubleRow
AF = mybir.ActivationFunctionType

_CACHE = {}


def _build_nc(sim=False, ablate=()):
    nc = bacc.Bacc("TRN2", target_bir_lowering=False, debug=False,
                   enable_asserts=False,
                   num_devices=1 if sim else NCORES)

    def inp(name, shape, dt=F8):
        return nc.dram_tensor(name, shape, dt, kind="ExternalInput").ap()

    x1f = inp("x1f", [128, CH, 32, H], BF)
    x1c = inp("x1c", [128, CH, 24, 192])
    x2b = inp("x2b", [128, CH, 24, 192])
    x3b = inp("x3b", [128, CH, 24, 192])
    wdT8 = inp("wdT8", [128, CH, 9, C])
    wqT8 = inp("wqT8", [128, CH, HID])
    wkT8 = inp("wkT8", [128, CH, HID])
    wvT8 = inp("wvT8", [128, CH, C])
    wca8 = inp("wca8", [128, CH, 9, C])
    wcb8 = inp("wcb8", [128, CH, 9, C])
    wfcb = inp("wfcb", [128, CH, C], BF)
    cq32 = inp("cq32", [64, CH, 1], F32)
    beff = inp("beff", [128, CH, 16], F32)
    ones8 = inp("ones8", [128, 2, 128])
    nb2 = inp("nb2", [128, 1], F32)   # exp bias (= ESHIFT)

    out = nc.dram_tensor("out", [CH, 128, 4 * RD, H], F32,
                         kind="ExternalOutput").ap()
    dbg = {}
    if "dbg" in ablate:
        for nm, shp, dt in (("d_qf", [64, CH, NL], F8),
                            ("d_kf", [64, CH, N], F8),
                            ("d_vt", [128, NJT, C], F8),
                            ("d_tp", [128, 2, NL], F8),
                            ("d_rb", [128, NL], BF),
                            ("d_feat", [128, CH, 2, NL], F8)):
            dbg[nm] = nc.dram_tensor(nm, shp, dt, kind="ExternalOutput").ap()

    # collective buffers: kf64 rows 0:64, vt rows 64:192
    kv_sh = [nc.dram_tensor(f"kv_sh{e}", [192, 1024], F8).ap()
             for e in range(2)]
    kv_fl = [nc.dram_tensor(f"kv_fl{e}", [NCORES * 192, 1024], F8,
                            addr_space="Shared").ap() for e in range(2)]
    rg = [list(range(NCORES))]

    with tile.TileContext(nc) as tc:
        with (
            tc.tile_pool(name="wsm", bufs=1) as wsm,
            tc.tile_pool(name="bigw", bufs=1) as bigw,
            tc.tile_pool(name="band", bufs=3) as band_pool,
            tc.tile_pool(name="x1fp", bufs=1) as x1fp,
            tc.tile_pool(name="xd", bufs=2) as xd_pool,
            tc.tile_pool(name="kvs", bufs=2) as kvs_pool,
            tc.tile_pool(name="kfull", bufs=2) as kfull_pool,
            tc.tile_pool(name="qf", bufs=1) as qf_pool,
            tc.tile_pool(name="tp", bufs=3) as tp_pool,
            tc.tile_pool(name="feat", bufs=1) as feat_pool,
            tc.tile_pool(name="rb", bufs=2) as rb_pool,
            tc.tile_pool(name="stg", bufs=2) as stg_pool,
            tc.tile_pool(name="p1", bufs=3, space="PSUM") as P1,
            tc.tile_pool(name="pf", bufs=2, space="PSUM") as PF,
            tc.tile_pool(name="p3", bufs=2, space="PSUM") as P3,
            tc.tile_pool(name="pd", bufs=1, space="PSUM") as PD,
        ):
            def load(pool, ap, tag, split=1, eng=None):
                t_ = pool.tile(ap.shape, ap.dtype, tag=tag)
                d1 = ap.shape[1]
                step = max(1, d1 // split)
                for ci, i in enumerate(range(0, d1, step)):
                    j = min(d1, i + step)
                    e = eng or (nc.sync if ci % 2 == 0 else nc.gpsimd)
                    e.dma_start(out=t_[:, i:j], in_=ap[:, i:j])
                return t_

            # priority loads: conv weights + x2 band first
            wdT_s = load(bigw, wdT8, "wdT", split=2)
            x2b_s = load(band_pool, x2b, "band", split=2)
            wqT_s = load(wsm, wqT8, "wqT", eng=nc.gpsimd)
            wkT_s = load(wsm, wkT8, "wkT", eng=nc.gpsimd)
            wvT_s = load(wsm, wvT8, "wvT", eng=nc.gpsimd)
            cq_s = load(wsm, cq32, "cq", eng=nc.gpsimd)
            ones_s = load(wsm, ones8, "ones", eng=nc.gpsimd)
            nb2_s = load(wsm, nb2, "nb2", eng=nc.gpsimd)
            beff_s = load(wsm, beff, "beff", eng=nc.gpsimd)
            x1c_s = load(band_pool, x1c, "band", split=2)
            x3b_s = load(band_pool, x3b, "band", split=2)

            # preload the Exp activation table off the critical path
            scr = wsm.tile([128, 2, 1], BF, tag="scr")
            nc.scalar.activation(scr[:], ones_s[:, :, 0], AF.Exp)

            def conv_down(band_s, name):
                """packed band [128,CH,24,192] -> xd [128,CH,NL] fp8."""
                xd_s = xd_pool.tile([128, CH, NL], F8, tag="xd", name=name)
                for m in range(CH):
                    ps = P1.tile([128, NL], F32, tag="p1")
                    for dy in range(3):
                        for dx in range(3):
                            tap = dy * 3 + dx
                            nc.tensor.matmul(
                                ps[:],
                                lhsT=wdT_s[:, :, tap, m * 128:(m + 1) * 128],
                                rhs=band_s[:, :, dy:24:3, dx:192:3],
                                start=(tap == 0), stop=(tap == 8),
                                perf_mode=DR)
                    # psum = 32*conv; store true-scale fp8
                    nc.scalar.activation(xd_s[:, m, :], ps[:], AF.Copy,
                                         scale=1.0 / WS)
                return xd_s

            # ---- branch k/v: own chunk only, then AllGather ----
            def kv_branch(e, band_s):
                xd_s = conv_down(band_s, f"x{e + 2}d")
                kf_s = kvs_pool.tile([64, CH, NL], F8, tag="kf")
                for h in range(CH):
                    psk = P1.tile([64, NL], F32, tag="p1")
                    nc.tensor.matmul(
                        psk[:],
                        lhsT=wkT_s[:, :, h * 64:(h + 1) * 64],
                        rhs=xd_s[:], start=True, stop=True, perf_mode=DR)
                    nc.vector.tensor_copy(kf_s[:, h, :], psk[:])
                vt_s = kvs_pool.tile([128, 4, C], F8, tag="vt")
                for j in range(4):
                    psv = P1.tile([128, C], F32, tag="p1")
                    nc.tensor.matmul(
                        psv[:],
                        lhsT=xd_s[:, :, j * 128:(j + 1) * 128],
                        rhs=wvT_s[:], start=True, stop=True, perf_mode=DR)
                    eng = nc.scalar if j % 2 == 0 else nc.vector
                    if eng is nc.scalar:
                        eng.activation(vt_s[:, j, :], psv[:], AF.Copy)
                    else:
                        eng.tensor_copy(vt_s[:, j, :], psv[:])
                nc.sync.dma_start(
                    out=kv_sh[e][0:64, :],
                    in_=kf_s.rearrange("p h n -> p (h n)"))
                nc.sync.dma_start(
                    out=kv_sh[e][64:192, :],
                    in_=vt_s.rearrange("p j c -> p (j c)"))
                if sim:
                    nc.gpsimd.dma_start(out=kv_fl[e][0:192, :],
                                        in_=kv_sh[e][:])
                else:
                    nc.gpsimd.collective_compute(
                        "AllGather", mybir.AluOpType.bypass,
                        replica_groups=rg,
                        ins=[kv_sh[e][:]], outs=[kv_fl[e][:]])

            kv_branch(0, x2b_s)

            # ---- x1 conv + q projection ----
            x1d_s = conv_down(x1c_s, "x1d")
            qf_s = qf_pool.tile([64, CH, NL], F8, tag="qf")
            for h in range(CH):
                psq = P1.tile([64, NL], F32, tag="p1")
                nc.tensor.matmul(
                    psq[:],
                    lhsT=wqT_s[:, :, h * 64:(h + 1) * 64],
                    rhs=x1d_s[:], start=True, stop=True, perf_mode=DR)
                nc.vector.tensor_scalar_add(qf_s[:, h, :], psq[:],
                                            cq_s[:, h, :])

            kv_branch(1, x3b_s)

            # remaining big loads (behind gathers in the queue)
            x1f_s = x1fp.tile([128, CH, 32, H], BF, tag="x1f")
            for ci, i in enumerate(range(0, 32, 4)):
                e = nc.sync if ci % 2 == 0 else nc.gpsimd
                e.dma_start(out=x1f_s[:, :, i:i + 4, :],
                            in_=x1f[:, :, i:i + 4, :])
            wca_s = load(bigw, wca8, "wca", split=2)
            wcb_s = load(bigw, wcb8, "wcb", split=2)
            wfc_s = load(wsm, wfcb, "wfc")

            # gather reads: stream chunks into SBUF as they land
            kfull = [kfull_pool.tile([64, CH, N], F8, tag="kfu",
                                     name=f"kfu{e}") for e in range(2)]
            vtfull = [kfull_pool.tile([128, NJT, C], F8, tag="vtf",
                                      name=f"vtf{e}") for e in range(2)]
            for e in range(2):
                flk = kv_fl[e].rearrange("(n r) (h w) -> n r h w",
                                         n=NCORES, h=CH)
                flw = kv_fl[e].rearrange("(n r) (j c) -> n r j c",
                                         n=NCORES, j=4)
                for n in range(NCORES):
                    eng = nc.sync if (e + n) % 2 == 0 else nc.gpsimd
                    eng.dma_start(
                        out=kfull[e][:, :, n * NL:(n + 1) * NL],
                        in_=flk[n, 0:64])
                    eng.dma_start(
                        out=vtfull[e][:, n * 4:(n + 1) * 4, :],
                        in_=flw[n, 64:192])

            # ---- fused-stage staging buffers + x1-only subgrid helper ----
            stg = [stg_pool.tile([128, CH, 16, H], F32, tag="stg",
                                 name=f"stg{half}") for half in range(2)]
            x1_sgs = []   # (half, m, ky, kx) with no conv tap
            for half in range(2):
                for m in range(CH):
                    for ky in range(4):
                        for kx in range(4):
                            if ky == 3 or kx == 3:
                                x1_sgs.append((half, m, ky, kx))

            def do_sg(half, m, ky, kx, pool, tag, use_act=False):
                """one fused-output subgrid: optional conv taps + x1 path."""
                y0 = 4 * half
                ps_full = pool.tile([128, 2, 4, HD], F32, tag=tag)
                ps_o = ps_full[:, 0]
                first = True
                if ky < 3 and kx < 3:
                    tap = ky * 3 + kx
                    for ws_, e in ((wca_s, 0), (wcb_s, 1)):
                        nc.tensor.matmul(
                            ps_o[:],
                            lhsT=ws_[:, :, tap, m * 128:(m + 1) * 128],
                            rhs=feat_s[:, :, e, y0 * HD:(y0 + 4) * HD],
                            start=first, stop=False, perf_mode=DR)
                        first = False
                r0 = 16 * half + ky
                for k in range(CH):
                    nc.tensor.matmul(
                        ps_o[:],
                        lhsT=wfc_s[:, k, m * 128:(m + 1) * 128],
                        rhs=x1f_s[:, k, r0:r0 + 13:4, kx:kx + 253:4],
                        start=first, stop=(k == CH - 1))
                    first = False
                dst = stg[half][:, m, ky:ky + 13:4, kx:kx + 253:4]
                bias_ap = beff_s[:, m, 4 * ky + kx:4 * ky + kx + 1]
                if use_act and (ky + kx + m) % 2 == 0:
                    nc.scalar.activation(dst, ps_o[:], AF.Identity,
                                         bias=bias_ap, scale=1.0 / WS)
                else:
                    nc.vector.tensor_scalar(dst, ps_o[:], 1.0 / WS, bias_ap,
                                            op0=mybir.AluOpType.mult,
                                            op1=mybir.AluOpType.add)

            # ---- attends ----
            feat_s = feat_pool.tile([128, CH, 2, NL], F8, tag="feat")
            x1q = list(x1_sgs)  # queue of x1-only subgrids to interleave
            for e in range(2):
                pf = [PF.tile([128, NL], F32, tag="pf", name=f"pf{e}_{m}")
                      for m in range(CH)]
                den = PD.tile([128, NL], F32, tag="pd", name=f"den{e}")
                for p in range(16):
                    t_pair = tp_pool.tile([128, 2, NL], F8, tag="tp")
                    for jj in range(2):
                        jt = 2 * p + jj
                        pa = P1.tile([128, NL], F32, tag="p1")
                        nc.tensor.matmul(
                            pa[:],
                            lhsT=kfull[e][:, :, jt * 128:(jt + 1) * 128],
                            rhs=qf_s[:], start=True, stop=True,
                            perf_mode=DR)
                        nc.scalar.activation(t_pair[:, jj, :], pa[:], AF.Exp,
                                             bias=nb2_s[:],
                                             scale=SCALE / (WS * WS))
                    for m in range(CH):
                        nc.tensor.matmul(
                            pf[m][:],
                            lhsT=vtfull[e][:, 2 * p:2 * p + 2,
                                           m * 128:(m + 1) * 128],
                            rhs=t_pair[:], start=(p == 0), stop=(p == 15),
                            perf_mode=DR)
                    nc.tensor.matmul(den[:], lhsT=ones_s[:], rhs=t_pair[:],
                                     start=(p == 0), stop=(p == 15),
                                     perf_mode=DR)
                    if "dbg" in ablate and e == 0 and p == 0:
                        nc.sync.dma_start(out=dbg["d_tp"][:], in_=t_pair[:])
                # den already holds the denominator on every partition
                rb_s = rb_pool.tile([128, NL], BF, tag="rb")
                with nc.allow_low_precision(reason="softmax denom recip"):
                    nc.vector.reciprocal(rb_s[:], den[:])
                if "dbg" in ablate and e == 0:
                    nc.sync.dma_start(out=dbg["d_rb"][:], in_=rb_s[:])
                for m in range(CH):
                    nc.vector.scalar_tensor_tensor(
                        feat_s[:, m, e, :], pf[m][:], 1.0 / WS, rb_s[:],
                        op0=mybir.AluOpType.mult, op1=mybir.AluOpType.mult)

            if "dbg" in ablate:
                nc.sync.dma_start(out=dbg["d_qf"][:], in_=qf_s[:])
                nc.sync.dma_start(out=dbg["d_kf"][:], in_=kfull[0][:])
                nc.sync.dma_start(out=dbg["d_vt"][:], in_=vtfull[0][:])
                nc.sync.dma_start(out=dbg["d_feat"][:], in_=feat_s[:])
            # ---- fused stage: remaining subgrids, stream rows out ----
            def store_rows(half, ky):
                for m in range(CH):
                    r0 = 16 * half + ky
                    eng = nc.sync if (ky + m) % 2 == 0 else nc.gpsimd
                    eng.dma_start(out=out[m, :, r0:r0 + 13:4, :],
                                  in_=stg[half][:, m, ky:ky + 13:4, :])

            for half in range(2):
                for ky in range(4):
                    for kx in range(4):
                        for m in range(CH):
                            if (kx + m) % 2 == 0:
                                do_sg(half, m, ky, kx, P1, 'p1', use_act=True)
                            else:
                                do_sg(half, m, ky, kx, P3, 'p3', use_act=True)
                    store_rows(half, ky)

    nc.compile()
    return nc


def _prep_inputs(x1, x2, x3, w_down, b_down, w_q, b_q, w_k, b_k, w_v, b_v,
                 w_up, b_up, w_fuse, b_fuse):
    f8 = ml_dtypes.float8_e4m3
    bf = ml_dtypes.bfloat16

    def to_tiles(a):
        # [C, ...] -> [128, CH, ...]
        return np.ascontiguousarray(
            a.reshape(CH, 128, *a.shape[1:]).transpose(
                1, 0, *range(2, a.ndim + 1)))

    rows24 = (np.arange(8)[:, None] * 4 + np.arange(3)).ravel()
    cols192 = (np.arange(64)[:, None] * 4 + np.arange(3)).ravel() - 1

    def band_packed(x, r):
        # only the rows/cols a stride-4 3x3 tap reads: [128,CH,24,192]
        rows = rows24 + 32 * r - 1
        rv = np.clip(rows, 0, H - 1)
        cv = np.clip(cols192, 0, H - 1)
        b = x[0][:, rv[:, None], cv[None, :]].astype(np.float32)
        b[:, rows < 0, :] = 0.0
        b[:, rows >= H, :] = 0.0
        b[:, :, cols192 < 0] = 0.0
        return to_tiles(b).astype(f8)

    wf = w_fuse[:, :, 0, 0]                      # [C, 3C]
    wdT = w_down.transpose(1, 2, 3, 0).reshape(C, 9, C)
    wq = w_q[:, :, 0, 0]
    wk = w_k[:, :, 0, 0]
    wv = w_v[:, :, 0, 0]
    wca = np.einsum('iokl,co->iklc', w_up, wf[:, :C],
                    optimize=True).reshape(C, 9, C)
    wcb = np.einsum('iokl,co->iklc', w_up, wf[:, C:2 * C],
                    optimize=True).reshape(C, 9, C)
    cq = WS * (wq @ b_down + b_q)                # [HID]
    cv = wv @ b_down + b_v                       # [C]
    beff = (b_fuse + wf[:, :C] @ b_up + wf[:, C:2 * C] @ b_up)  # [C]
    bcorr = np.einsum('itc,i->tc', wca + wcb, cv)  # [9, C]
    beff_sg = np.broadcast_to(beff[:, None], (C, 16)).copy()
    for ky in range(3):
        for kx in range(3):
            beff_sg[:, 4 * ky + kx] += bcorr[ky * 3 + kx]

    shared = {
        "wdT8": to_tiles(wdT * WS).astype(f8),
        "wqT8": to_tiles((wq.T * WS).copy()).astype(f8),
        "wkT8": to_tiles((wk.T * WS).copy()).astype(f8),
        "wvT8": to_tiles((wv.T * WS).copy()).astype(f8),
        "wca8": to_tiles(wca * WS).astype(f8),
        "wcb8": to_tiles(wcb * WS).astype(f8),
        "wfcb": to_tiles((wf[:, 2 * C:].T * WS).copy()).astype(bf),
        "cq32": np.ascontiguousarray(
            cq.reshape(CH, 64).T).reshape(64, CH, 1).astype(np.float32),
        "beff": to_tiles(beff_sg).astype(np.float32),
        "ones8": np.ones((128, 2, 128), f8),
        "nb2": np.full((128, 1), ESHIFT, np.float32),
    }
    in_maps = []
    for r in range(NCORES):
        m = dict(shared)
        m["x1f"] = to_tiles(
            x1[0, :, 32 * r:32 * r + 32, :].astype(np.float32)).astype(bf)
        m["x1c"] = band_packed(x1, r)
        m["x2b"] = band_packed(x2, r)
        m["x3b"] = band_packed(x3, r)
        in_maps.append(m)
    return in_maps


def kernel(**inputs):
    inputs = {k: np.asarray(v) for k, v in inputs.items()}
    in_maps = _prep_inputs(**inputs)
    if "nc" not in _CACHE:
        _CACHE["nc"] = _build_nc()
    res = run_bass_kernel_spmd(_CACHE["nc"], in_maps,
                               core_ids=list(range(NCORES)))
    out = np.empty((1, C, H, H), np.float32)
    for r in range(NCORES):
        band = res.results[r]["out"].reshape(C, 4 * RD, H)
        out[0, :, 32 * r:32 * r + 32, :] = band
    return out
